# revision 5
# baseline (speedup 1.0000x reference)
"""Causal ALiBi attention (GQA) on 8 Trainium2 NeuronCores.

Sharding: 2 (batch) x 4 (head groups). Core (b, g) computes batch b, query
heads 4g..4g+3 and kv head g, producing a partial output (its heads'
contribution to the out-projection). Host sums the 4 partials per batch and
adds bo.

Per-core kernel (all matmuls bf16 with fp32 PSUM accumulation):
  - projections: out[f_tile, t] = sum_d W^T[d, f] * x^T[d, t]; an appended
    ones-row of x^T folds the linear biases in exactly. SCALE is folded into
    Wq/bq on the host.
  - softmax without a max-reduction: the exponent S + slope*(j - qpos) - 12
    is bounded above (causal => j <= qpos, |S| << 12 margin), so exp cannot
    overflow, and per-row shifts cancel in the normalization. The exponent is
    split as
        slope*(j - qpos) = slope*((j mod 512) - 511)       -> folded into V
                         + slope*(512c + 511 - qpos)       -> ACT bias [128,1]
    making exp a single activation per 512-wide chunk (per-partition bias),
    while the pure-k factor rides on V's rows:
    V'[j,:] = e^{slope*((j mod 512)-511)} * V[j,:]. Both factors stay in
    range; entries that underflow are < e^-40 of their row max.
  - causal masking: above-diagonal k-tiles are skipped outright; only the
    diagonal 128x128 tile needs a triangular 0/1 multiply.
  - P^T via PE transposes; the AV matmul rhs is [V' | E-column], so PSUM
    column 128 accumulates the softmax denominator l_q for free, and the
    PSUM->SBUF copy is a tensor_scalar multiply by 1/l (free normalization).
  - attn output is PE-transposed back to [f, t]; out-projection contracts f
    with Wo^T chunks; partial out rows are DMA'd to HBM as bf16.
"""

import numpy as np
import ml_dtypes

import concourse.bass as bass
import concourse.tile as tile
from concourse import bacc, mybir
from concourse.bass_utils import run_bass_kernel_spmd
from concourse.masks import make_identity

B, S, D = 2, 2048, 2048
H, KV, DH = 16, 4, 128
SCALE = 1.0 / np.sqrt(DH)
NCORES = 8
NG = 4            # head groups (= kv heads); one per core within a batch
HPG = H // NG     # query heads per group (4)
FPG = HPG * DH    # features per group (512)
P = 128           # partitions
QT = S // P       # q tiles (16)
CW = 512          # k chunk width
NCH = S // CW     # k chunks (4)
KPC = CW // P     # k-tiles per chunk (4)
KA = D + P        # contraction rows incl. padded ones-row block (2176)
KCH = KA // P     # contraction chunks (17)
VW = P + 4        # padded AV rhs width (129 used)
EXP_SHIFT = 12.0  # constant exponent headroom margin

_BF16 = ml_dtypes.bfloat16
_COMPILED = {}    # causal -> compiled Bacc program
LAST_RUN = {}     # timing info from the most recent run


def _build(causal: bool):
    nc = bacc.Bacc("TRN2", target_bir_lowering=False, debug=False,
                   num_devices=NCORES)
    dt = mybir.dt

    xTa = nc.dram_tensor("xTa", [KA, S], dt.bfloat16, kind="ExternalInput")
    wqTa = nc.dram_tensor("wqTa", [KA, FPG], dt.bfloat16, kind="ExternalInput")
    wkTa = nc.dram_tensor("wkTa", [KA, DH], dt.bfloat16, kind="ExternalInput")
    wvTa = nc.dram_tensor("wvTa", [KA, DH], dt.bfloat16, kind="ExternalInput")
    woT = nc.dram_tensor("woT", [FPG, D], dt.bfloat16, kind="ExternalInput")
    evcol = nc.dram_tensor("evcol", [P, QT, HPG], dt.float32,
                           kind="ExternalInput")
    biast = nc.dram_tensor("biast", [P, QT, NCH, HPG], dt.float32,
                           kind="ExternalInput")
    mm = nc.dram_tensor("mm", [P, KPC, HPG, P], dt.bfloat16,
                        kind="ExternalInput")
    out = nc.dram_tensor("out", [S, D], dt.bfloat16, kind="ExternalOutput")

    with tile.TileContext(nc) as tc:
        with (
            tc.tile_pool(name="persist", bufs=1) as persist,
            tc.tile_pool(name="psum", bufs=3, space="PSUM") as psum,
            tc.tile_pool(name="trps", bufs=2, space="PSUM") as trps,
        ):
            # ---- persistent tiles ----
            wo_sb = persist.tile([P, FPG // P, D], dt.bfloat16)
            nc.sync.dma_start(wo_sb[:], woT.rearrange("(c p) f -> p c f", p=P))
            ev_sb = persist.tile([P, QT, HPG], dt.float32)
            nc.sync.dma_start(ev_sb[:], evcol[:])
            bias_sb = persist.tile([P, QT, NCH, HPG], dt.float32)
            nc.sync.dma_start(bias_sb[:], biast[:])
            mm_sb = persist.tile([P, KPC, HPG, P], dt.bfloat16)
            nc.sync.dma_start(mm_sb[:], mm[:])
            ident = persist.tile([P, P], dt.bfloat16)
            make_identity(nc, ident[:])

            qT = persist.tile([P, HPG, S], dt.bfloat16)
            kT = persist.tile([P, S], dt.bfloat16)
            vT = persist.tile([P, S], dt.bfloat16)
            vaug = persist.tile([P, HPG, QT, VW], dt.bfloat16)
            vplain = persist.tile([P, QT, VW], dt.bfloat16)
            aT = persist.tile([P, HPG, S], dt.bfloat16)

            # ---------------- phase 1: projections ----------------
            with tc.tile_pool(name="wpool", bufs=1) as wpool:
                xTa_sb = wpool.tile([P, KCH, S], dt.bfloat16)
                nc.sync.dma_start(xTa_sb[:],
                                  xTa.rearrange("(c p) t -> p c t", p=P))
                wq_sb = wpool.tile([P, KCH, FPG], dt.bfloat16)
                nc.sync.dma_start(wq_sb[:],
                                  wqTa.rearrange("(c p) f -> p c f", p=P))
                wk_sb = wpool.tile([P, KCH, DH], dt.bfloat16)
                nc.sync.dma_start(wk_sb[:],
                                  wkTa.rearrange("(c p) f -> p c f", p=P))
                wv_sb = wpool.tile([P, KCH, DH], dt.bfloat16)
                nc.sync.dma_start(wv_sb[:],
                                  wvTa.rearrange("(c p) f -> p c f", p=P))

                def proj(dst, wtile, fofs):
                    for t in range(S // CW):
                        ps = psum.tile([P, CW], dt.float32, tag="acc")
                        for c in range(KCH):
                            nc.tensor.matmul(
                                ps[:],
                                wtile[:, c, fofs * P:(fofs + 1) * P],
                                xTa_sb[:, c, t * CW:(t + 1) * CW],
                                start=(c == 0), stop=(c == KCH - 1))
                        nc.any.tensor_copy(dst[:, t * CW:(t + 1) * CW], ps[:])

                proj(kT, wk_sb, 0)
                proj(vT, wv_sb, 0)
                for h in range(HPG):
                    proj(qT[:, h], wq_sb, h)

            # ------------- phase 2: attention + out-projection -------------
            with (
                tc.tile_pool(name="work", bufs=2) as work,
                tc.tile_pool(name="small", bufs=4) as small,
            ):
                # V token-major; fold E into V' and append the E column.
                nc.vector.memset(vplain[:, :, P], 1.0)
                for kt in range(QT):
                    vps = trps.tile([P, P], dt.bfloat16, tag="tr")
                    nc.tensor.transpose(vps[:], vT[:, kt * P:(kt + 1) * P],
                                        ident[:])
                    nc.any.tensor_copy(vplain[:, kt, 0:P], vps[:])
                    for h in range(HPG):
                        nc.vector.tensor_scalar_mul(
                            vaug[:, h, kt, 0:P], vplain[:, kt, 0:P],
                            ev_sb[:, kt, h:h + 1])
                        nc.vector.tensor_copy(
                            vaug[:, h, kt, P:P + 1], ev_sb[:, kt, h:h + 1])

                for h in range(HPG):
                    for qi in range(QT):
                        dc = qi // KPC      # diagonal chunk
                        dkt = qi % KPC      # diagonal k-tile within chunk
                        nch = dc + 1 if causal else NCH
                        nkt = qi + 1 if causal else QT
                        pexp = work.tile([P, S], dt.bfloat16, tag="pexp")
                        for c in range(nch):
                            w = (dkt + 1) * P if (causal and c == dc) else CW
                            sps = psum.tile([P, CW], dt.float32, tag="acc")
                            nc.tensor.matmul(
                                sps[:, 0:w],
                                qT[:, h, qi * P:(qi + 1) * P],
                                kT[:, c * CW:c * CW + w],
                                start=True, stop=True)
                            nc.scalar.activation(
                                pexp[:, c * CW:c * CW + w], sps[:, 0:w],
                                mybir.ActivationFunctionType.Exp,
                                bias=bias_sb[:, qi, c, h:h + 1], scale=1.0)
                        if causal:
                            # diag-chunk tiles: alibi(+mask) multiplicative
                            for kt in range(dc * KPC, qi + 1):
                                dd = KPC - 1 - (qi - kt)
                                nc.vector.tensor_mul(
                                    pexp[:, kt * P:(kt + 1) * P],
                                    pexp[:, kt * P:(kt + 1) * P],
                                    mm_sb[:, dd, h])
                        avps = psum.tile([P, VW], dt.float32, tag="av")
                        pts = work.tile([P, QT, P], dt.bfloat16, tag="pts")
                        for kt in range(nkt):
                            ptp = trps.tile([P, P], dt.bfloat16, tag="tr")
                            nc.tensor.transpose(
                                ptp[:], pexp[:, kt * P:(kt + 1) * P], ident[:])
                            nc.any.tensor_copy(pts[:, kt], ptp[:])
                            vrhs = (vplain[:, kt, 0:P + 1]
                                    if causal and kt >= dc * KPC
                                    else vaug[:, h, kt, 0:P + 1])
                            nc.tensor.matmul(
                                avps[:, 0:P + 1], pts[:, kt], vrhs,
                                start=(kt == 0), stop=(kt == nkt - 1))
                        rec = small.tile([P, 1], dt.float32, tag="rec")
                        nc.vector.reciprocal(rec[:], avps[:, P:P + 1])
                        anorm = small.tile([P, P], dt.bfloat16, tag="anorm")
                        nc.vector.tensor_scalar_mul(anorm[:], avps[:, 0:P],
                                                    rec[:])
                        atp = trps.tile([P, P], dt.bfloat16, tag="tr")
                        nc.tensor.transpose(atp[:], anorm[:], ident[:])
                        nc.any.tensor_copy(aT[:, h, qi * P:(qi + 1) * P],
                                           atp[:])

                # out-projection
                for t in range(QT):
                    osb = work.tile([P, D], dt.bfloat16, tag="osb")
                    for n in range(D // CW):
                        ops = psum.tile([P, CW], dt.float32, tag="acc")
                        for f in range(FPG // P):
                            nc.tensor.matmul(
                                ops[:],
                                aT[:, f, t * P:(t + 1) * P],
                                wo_sb[:, f, n * CW:(n + 1) * CW],
                                start=(f == 0), stop=(f == FPG // P - 1))
                        nc.any.tensor_copy(osb[:, n * CW:(n + 1) * CW], ops[:])
                    nc.sync.dma_start(out[t * P:(t + 1) * P, :], osb[:])

    nc.compile()
    return nc


def _get_program(causal: bool):
    if causal not in _COMPILED:
        _COMPILED[causal] = _build(causal)
    return _COMPILED[causal]


def _detect_mask(attention_mask: np.ndarray) -> bool:
    am = np.asarray(attention_mask).reshape(S, S)
    if not am.any():
        return False
    tri = np.tril(np.ones((S, S), dtype=bool))
    if np.all(am[tri] == 0.0) and np.all(am[~tri] <= -1e8):
        return True
    raise ValueError("kernel supports causal (0/-1e9) or all-zero masks only")


def _prep_core_inputs(hidden_states, Wq, bq, Wk, bk, Wv, bv, Wo,
                      alibi_slopes, causal):
    """Build the 8 per-core input maps (host-side shard + fold)."""
    j = np.arange(S)
    p = np.arange(P)
    qi = np.arange(QT)
    c = np.arange(NCH)
    kl = np.arange(P)
    in_maps = [None] * NCORES
    for b in range(B):
        xTa = np.zeros((KA, S), dtype=_BF16)
        xTa[:D] = np.ascontiguousarray(hidden_states[b].T).astype(_BF16)
        xTa[D] = 1.0
        for g in range(NG):
            sl = np.asarray(alibi_slopes[g * HPG:(g + 1) * HPG], np.float64)
            wqTa = np.zeros((KA, FPG), dtype=_BF16)
            wqTa[:D] = (SCALE * Wq[g * FPG:(g + 1) * FPG, :].T).astype(_BF16)
            wqTa[D] = (SCALE * bq[g * FPG:(g + 1) * FPG]).astype(_BF16)
            wkTa = np.zeros((KA, DH), dtype=_BF16)
            wkTa[:D] = Wk[g * DH:(g + 1) * DH, :].T.astype(_BF16)
            wkTa[D] = bk[g * DH:(g + 1) * DH].astype(_BF16)
            wvTa = np.zeros((KA, DH), dtype=_BF16)
            wvTa[:D] = Wv[g * DH:(g + 1) * DH, :].T.astype(_BF16)
            wvTa[D] = bv[g * DH:(g + 1) * DH].astype(_BF16)
            woT = np.ascontiguousarray(
                Wo[:, g * FPG:(g + 1) * FPG].T).astype(_BF16)
            # E column e^{slope*((j mod 512) - 511)} laid out [p, ktile, h]
            ev = np.exp(sl[None, :] * ((j % CW)[:, None] - (CW - 1.0)))
            evcol = np.ascontiguousarray(
                ev.reshape(QT, P, HPG).transpose(1, 0, 2)).astype(np.float32)
            # ACT bias [p, qi, c, h]
            if causal:
                val = (sl[None, None, None, :]
                       * (CW * c[None, None, :, None] + (CW - 1.0)
                          - P * qi[None, :, None, None]
                          - p[:, None, None, None]) - EXP_SHIFT)
                # diagonal chunk: flat -12 bias (alibi+mask move into mm)
                val = np.array(val)
                for q_ in range(QT):
                    val[:, q_, q_ // KPC, :] = -EXP_SHIFT
            else:
                val = np.broadcast_to(
                    sl[None, None, None, :]
                    * (CW * c[None, None, :, None] + (CW - 1.0) - (S - 1.0))
                    - EXP_SHIFT, (P, QT, NCH, HPG))
            biast = np.ascontiguousarray(val).astype(np.float32)
            # mm[p, dd, h, kl] = e^{slope*(128*(dd-3) + kl - p)}, triangular
            # mask at dd=3 (the diagonal tile)
            dd = np.arange(KPC)
            mmv = np.exp(np.minimum(
                sl[None, None, :, None]
                * (P * (dd[None, :, None, None] - (KPC - 1.0))
                   + kl[None, None, None, :] - p[:, None, None, None]), 0.0))
            mmv[:, KPC - 1, :, :] *= (kl[None, None, :]
                                      <= p[:, None, None])
            mmval = np.ascontiguousarray(mmv).astype(_BF16)
            in_maps[b * NG + g] = {
                "xTa": xTa, "wqTa": wqTa, "wkTa": wkTa, "wvTa": wvTa,
                "woT": woT, "evcol": evcol, "biast": biast, "mm": mmval,
            }
    return in_maps


def kernel(hidden_states, attention_mask, Wq, bq, Wk, bk, Wv, bv, Wo, bo,
           alibi_slopes):
    import time
    causal = _detect_mask(attention_mask)
    nc = _get_program(causal)
    in_maps = _prep_core_inputs(
        np.asarray(hidden_states, np.float32), np.asarray(Wq, np.float32),
        np.asarray(bq, np.float32), np.asarray(Wk, np.float32),
        np.asarray(bk, np.float32), np.asarray(Wv, np.float32),
        np.asarray(bv, np.float32), np.asarray(Wo, np.float32),
        np.asarray(alibi_slopes, np.float32), causal)
    t0 = time.perf_counter()
    res = run_bass_kernel_spmd(nc, in_maps, list(range(NCORES)))
    t1 = time.perf_counter()
    LAST_RUN["wall_s"] = t1 - t0
    out = np.zeros((B, S, D), dtype=np.float32)
    for b in range(B):
        for g in range(NG):
            out[b] += np.asarray(res.results[b * NG + g]["out"],
                                 dtype=np.float32)
        out[b] += np.asarray(bo, np.float32)[None, :]
    return out
